# revision 7
# baseline (speedup 1.0000x reference)
"""Group-wise correlation cost volume kernel for Trainium2 (8 NeuronCores).

Computes, for inputs left_gwc/right_gwc [B=4, C=320, H=64, W=128] f32:
  cv[b, g, d, h, w] = mean_c( lg[b, g, c, h, w] * rg[b, g, c, h, w-d] )  (0 where w < d)
with G=40 groups (Cg=8 channels each), D=48 disparities.
Output: [4, 40, 48, 64, 128] f32.

Per core (160 (b,g) pairs sharded 20/core across 8 cores):
 - SBUF layout: partitions = (c4, h32); free dim packs 40 slabs
   (j=pair%4, hb=h-block, i=pair//4) x W.  Right features carry 48 zero
   columns of left-padding so the w<d mask falls out as exact zeros, in two
   parity copies (data offset 48 / 47) keeping shifted fp16 reads 4B-aligned
   for the DVE 2x mode.  Host pre-permutes/casts inputs into this layout.
 - Products lg*shift(rg) in fp16 on VectorE (2 big tensor_mul per d).
 - Channel mean on TensorE: static 0.125*block-diag(h32) weights [128,32],
   two accumulating matmuls contract (c4,h32)->h32; 4-way column tiling
   (tile_position (0,32j)) fills all 128 PSUM partitions.
 - PSUM -> SBUF on ScalarE; one HWDGE DMA per d-plane into a scratch-layout
   DRAM output [48, 128, 1280] that the host un-permutes.
"""

import os
import sys

import numpy as np

for _p in ("/opt/trn_rl_repo", "/root/.axon_site/_ro/trn_rl_repo"):
    if os.path.isdir(_p) and _p not in sys.path:
        sys.path.insert(0, _p)

from contextlib import ExitStack

import concourse.bass as bass
import concourse.mybir as mybir
from concourse import bacc
from concourse.bass_utils import run_bass_kernel_spmd
from concourse.tile import TileContext

B, CH, H, W = 4, 320, 64, 128
G, D, CG = 40, 48, 8
NCORES = 8
PAIRS = (B * G) // NCORES  # 20 (b,g) pairs per core
JW = 4                     # column-tile groups
IW = PAIRS // JW           # 5
S = PAIRS * 2              # 40 slabs: s = j*10 + hb*5 + i
PADX = 176                 # 48 pad + 128 data columns of right features
FP16 = mybir.dt.float16
FP32 = mybir.dt.float32

_built = None


def _build():
    nc = bacc.Bacc(
        "TRN2",
        target_bir_lowering=False,
        debug=False,
        num_devices=NCORES,
    )
    lgd = nc.declare_dram_parameter("lgd", [2, 128, S * W], FP16, isOutput=False)
    rgd = nc.declare_dram_parameter("rgd", [2, 2, 128, S * PADX], FP16, isOutput=False)
    wt = nc.declare_dram_parameter("wt", [128, 32], FP16, isOutput=False)
    out = nc.declare_dram_parameter("out", [D, 128, 2 * IW * W], FP32, isOutput=True)

    with TileContext(nc) as tc, ExitStack() as ctx:
        cpool = ctx.enter_context(tc.tile_pool(name="cpool", bufs=1))
        ipool = ctx.enter_context(tc.tile_pool(name="ipool", bufs=1))
        ppool = ctx.enter_context(tc.tile_pool(name="ppool", bufs=6))
        spool = ctx.enter_context(tc.tile_pool(name="spool", bufs=3))
        qpool = ctx.enter_context(tc.tile_pool(name="qpool", bufs=2, space="PSUM"))

        wt_sb = cpool.tile([128, 32], FP16)
        nc.sync.dma_start(out=wt_sb[:], in_=wt[:])

        # Load even-parity tiles first (needed for d=0) and split across the
        # two HWDGE engines so the two queues run in parallel.
        lg_sb = []  # [k] -> [128, S*W] fp16
        rg_sb = [[None, None], [None, None]]  # [k][parity] -> [128, S*PADX]
        for k in range(2):
            eng = nc.sync if k == 0 else nc.scalar
            lgt = ipool.tile([128, S * W], FP16, name=f"lg{k}", tag=f"lg{k}")
            eng.dma_start(out=lgt[:], in_=lgd[k])
            lg_sb.append(lgt)
            rgt = ipool.tile([128, S * PADX], FP16, name=f"rg{k}0", tag=f"rg{k}0")
            eng.dma_start(out=rgt[:], in_=rgd[k, 0])
            rg_sb[k][0] = rgt
        for k in range(2):
            eng = nc.sync if k == 0 else nc.scalar
            rgt = ipool.tile([128, S * PADX], FP16, name=f"rg{k}1", tag=f"rg{k}1")
            eng.dma_start(out=rgt[:], in_=rgd[k, 1])
            rg_sb[k][1] = rgt

        lg_v = [t.rearrange("p (s w) -> p s w", w=W) for t in lg_sb]
        rg_v = [
            [t.rearrange("p (s x) -> p s x", x=PADX) for t in row] for row in rg_sb
        ]

        # N<=512 blocks over the 10 slabs (1280 elems) owned by one column group
        UBLK = ((0, 512), (512, 512), (1024, 256))

        for d in range(D):
            par = d & 1
            o = (48 - d) if par == 0 else (47 - d)

            prods = []
            for k in range(2):
                pr = ppool.tile([128, S * W], FP16, name="pr", tag="pr")
                nc.vector.tensor_mul(
                    pr.rearrange("p (s w) -> p s w", w=W),
                    lg_v[k],
                    rg_v[k][par][:, :, o : o + W],
                )
                prods.append(pr)

            ps = qpool.tile([128, 1280], FP32, name="ps", tag="ps")
            for j in range(JW):
                for off, n in UBLK:
                    base = j * 10 * W + off
                    for k in range(2):
                        nc.tensor.matmul(
                            ps[32 * j : 32 * j + 32, off : off + n],
                            wt_sb[:],
                            prods[k][:, base : base + n],
                            start=(k == 0),
                            stop=(k == 1),
                            tile_position=(0, 32 * j),
                        )

            st = spool.tile([128, 1280], FP32, name="st", tag="st")
            nc.scalar.copy(st[:], ps[:])
            nc.sync.dma_start(out=out[d], in_=st[:])

    nc.compile()
    return nc


def _get_built():
    global _built
    if _built is None:
        _built = _build()
    return _built


def _make_weights():
    wt = np.zeros((128, 32), np.float16)
    for cc in range(4):
        wt[cc * 32 : (cc + 1) * 32, :] = np.eye(32, dtype=np.float16) * np.float16(
            0.125
        )
    return wt


def _prep_core(arr):
    """[20, 8, 64, 128] f32 -> ([2,128,S*W] fp16 partition layout)."""
    x = arr.reshape(IW, JW, 2, 4, 2, 32, W)  # i j k cc hb hh w
    x = np.transpose(x, (2, 3, 5, 1, 4, 0, 6))  # k cc hh j hb i w
    return np.ascontiguousarray(x.reshape(2, 128, S * W).astype(np.float16))


def run(inputs, trace=False):
    left = np.ascontiguousarray(np.asarray(inputs["left_gwc"], dtype=np.float32))
    right = np.ascontiguousarray(np.asarray(inputs["right_gwc"], dtype=np.float32))
    lgf = left.reshape(B * G, CG, H, W)
    rgf = right.reshape(B * G, CG, H, W)
    wt = _make_weights()

    in_maps = []
    for c in range(NCORES):
        sl = slice(c * PAIRS, (c + 1) * PAIRS)
        lg_dev = _prep_core(lgf[sl])
        r = _prep_core(rgf[sl]).reshape(2, 128, S, W)
        rg_dev = np.zeros((2, 2, 128, S, PADX), np.float16)
        rg_dev[:, 0, :, :, 48 : 48 + W] = r
        rg_dev[:, 1, :, :, 47 : 47 + W] = r
        in_maps.append(
            {
                "lgd": lg_dev,
                "rgd": np.ascontiguousarray(rg_dev.reshape(2, 2, 128, S * PADX)),
                "wt": wt,
            }
        )

    nc = _get_built()
    res = run_bass_kernel_spmd(nc, in_maps, list(range(NCORES)), trace=trace)

    full = np.empty((B * G, D, H, W), np.float32)
    for c in range(NCORES):
        r = res.results[c]["out"].reshape(D, JW, 32, 2, IW, W)  # d j hh hb i w
        r = np.transpose(r, (4, 1, 0, 3, 2, 5))  # i j d hb hh w
        full[c * PAIRS : (c + 1) * PAIRS] = r.reshape(PAIRS, D, H, W)
    return full.reshape(B, G, D, H, W), res.exec_time_ns


def kernel(**inputs):
    out, _ = run(inputs)
    return out


# revision 11
# speedup vs baseline: 1.1329x; 1.1329x over previous
"""Group-wise correlation cost volume kernel for Trainium2 (8 NeuronCores).

Computes, for inputs left_gwc/right_gwc [B=4, C=320, H=64, W=128] f32:
  cv[b, g, d, h, w] = mean_c( lg[b, g, c, h, w] * rg[b, g, c, h, w-d] )  (0 where w < d)
with G=40 groups (Cg=8 channels each), D=48 disparities.
Output: [4, 40, 48, 64, 128] f32.

Per core (160 (b,g) pairs sharded 20/core across 8 cores):
 - SBUF layout: partitions = (c4, h32); free dim packs 40 slabs
   (j=pair%4, hb=h-block, i=pair//4) x W.  Right features carry 48 zero
   columns of left-padding so the w<d mask falls out as exact zeros, in two
   parity copies (data offset 48 / 47) keeping shifted fp16 reads 4B-aligned
   for the DVE 2x mode.  Host pre-permutes/casts inputs into this layout.
 - Products lg*shift(rg) in fp16 on VectorE (2 big tensor_mul per d).
 - Channel mean on TensorE: static 0.125*block-diag(h32) weights [128,32],
   two accumulating matmuls contract (c4,h32)->h32; 4-way column tiling
   (tile_position (0,32j)) fills all 128 PSUM partitions.
 - PSUM -> SBUF on ScalarE; one HWDGE DMA per d-plane into a scratch-layout
   DRAM output [48, 128, 1280] that the host un-permutes.
"""

import os
import sys

import numpy as np

for _p in ("/opt/trn_rl_repo", "/root/.axon_site/_ro/trn_rl_repo"):
    if os.path.isdir(_p) and _p not in sys.path:
        sys.path.insert(0, _p)

from contextlib import ExitStack

import concourse.bass as bass
import concourse.mybir as mybir
from concourse import bacc
from concourse.bass_utils import run_bass_kernel_spmd
from concourse.tile import TileContext

B, CH, H, W = 4, 320, 64, 128
G, D, CG = 40, 48, 8
NCORES = 8
PAIRS = (B * G) // NCORES  # 20 (b,g) pairs per core
JW = 4                     # column-tile groups
IW = PAIRS // JW           # 5
S = PAIRS * 2              # 40 slabs: s = j*10 + hb*5 + i
PADX = 176                 # 48 pad + 128 data columns of right features
FP16 = mybir.dt.float16
FP32 = mybir.dt.float32

_built = None


def _build():
    nc = bacc.Bacc(
        "TRN2",
        target_bir_lowering=False,
        debug=False,
        num_devices=NCORES,
    )
    lgd = nc.declare_dram_parameter("lgd", [2, 128, S * W], FP16, isOutput=False)
    rgd = nc.declare_dram_parameter("rgd", [2, 2, 128, S * PADX], FP16, isOutput=False)
    wt = nc.declare_dram_parameter("wt", [128, 32], FP16, isOutput=False)
    out = nc.declare_dram_parameter("out", [D, 128, 2 * IW * W], FP32, isOutput=True)

    with TileContext(nc) as tc, ExitStack() as ctx:
        cpool = ctx.enter_context(tc.tile_pool(name="cpool", bufs=1))
        ipool = ctx.enter_context(tc.tile_pool(name="ipool", bufs=1))
        ppool = ctx.enter_context(tc.tile_pool(name="ppool", bufs=6))
        spool = ctx.enter_context(tc.tile_pool(name="spool", bufs=3))
        qpool = ctx.enter_context(tc.tile_pool(name="qpool", bufs=2, space="PSUM"))

        wt_sb = cpool.tile([128, 32], FP16)
        nc.sync.dma_start(out=wt_sb[:], in_=wt[:])

        # Load even-parity tiles, split across the two HWDGE engines so the
        # queues run in parallel; derive the odd-parity (1-shifted) copy
        # on-chip on ScalarE instead of spending input DMA bandwidth on it.
        lg_sb = []  # [k] -> [128, S*W] fp16
        rg_sb = [[None, None], [None, None]]  # [k][parity] -> [128, S*PADX]
        for k in range(2):
            eng = nc.sync if k == 0 else nc.scalar
            lgt = ipool.tile([128, S * W], FP16, name=f"lg{k}", tag=f"lg{k}")
            eng.dma_start(out=lgt[:], in_=lgd[k])
            lg_sb.append(lgt)
            rgt = ipool.tile([128, S * PADX], FP16, name=f"rg{k}0", tag=f"rg{k}0")
            eng.dma_start(out=rgt[:], in_=rgd[k, 0])
            rg_sb[k][0] = rgt
        for k in range(2):
            rgo = ipool.tile([128, S * PADX], FP16, name=f"rg{k}1", tag=f"rg{k}1")
            nc.scalar.copy(rgo[:, 0 : S * PADX - 1], rg_sb[k][0][:, 1 : S * PADX])
            nc.scalar.mul(rgo[:, S * PADX - 1 : S * PADX], rgo[:, S * PADX - 1 : S * PADX], 0.0)
            rg_sb[k][1] = rgo

        # Zero-init the product pool slots once (hidden under the input-DMA
        # wait) so the skipped w<d regions only ever hold finite values.
        for z in range(6):
            przi = ppool.tile([128, S * W], FP16, name=f"przi{z}", tag="pr")
            nc.vector.memset(przi[:], 0.0)

        lg_v = [t.rearrange("p (s w) -> p s w", w=W) for t in lg_sb]
        rg_v = [
            [t.rearrange("p (s x) -> p s x", x=PADX) for t in row] for row in rg_sb
        ]

        # N<=512 blocks over the 10 slabs (1280 elems) owned by one column group
        UBLK = ((0, 512), (512, 512), (1024, 256))

        for d in range(D):
            par = d & 1
            o = (48 - d) if par == 0 else (47 - d)

            # w < d is structurally zero: skip those products (round the
            # slice start down to even to keep fp16 reads 4B-aligned; the
            # skipped region carries stale garbage that gets zeroed on the
            # staged output below).
            a = d & ~1
            prods = []
            for k in range(2):
                pr = ppool.tile([128, S * W], FP16, name="pr", tag="pr")
                nc.vector.tensor_mul(
                    pr.rearrange("p (s w) -> p s w", w=W)[:, :, a:],
                    lg_v[k][:, :, a:],
                    rg_v[k][par][:, :, o + a : o + W],
                )
                prods.append(pr)

            ps = qpool.tile([128, 1280], FP32, name="ps", tag="ps")
            for j in range(JW):
                for off, n in UBLK:
                    base = j * 10 * W + off
                    for k in range(2):
                        nc.tensor.matmul(
                            ps[32 * j : 32 * j + 32, off : off + n],
                            wt_sb[:],
                            prods[k][:, base : base + n],
                            start=(k == 0),
                            stop=(k == 1),
                            tile_position=(0, 32 * j),
                        )

            st = spool.tile([128, 1280], FP32, name="st", tag="st")
            nc.scalar.copy(st[:], ps[:])
            if d > 0:
                strip = st.rearrange("p (u w) -> p u w", w=W)[:, :, 0:d]
                nc.scalar.mul(strip, strip, 0.0)
            nc.sync.dma_start(out=out[d], in_=st[:])

    nc.compile()
    return nc


def _get_built():
    global _built
    if _built is None:
        _built = _build()
    return _built


def _make_weights():
    wt = np.zeros((128, 32), np.float16)
    for cc in range(4):
        wt[cc * 32 : (cc + 1) * 32, :] = np.eye(32, dtype=np.float16) * np.float16(
            0.125
        )
    return wt


def _prep_core(arr):
    """[20, 8, 64, 128] f32 -> ([2,128,S*W] fp16 partition layout)."""
    x = arr.reshape(IW, JW, 2, 4, 2, 32, W)  # i j k cc hb hh w
    x = np.transpose(x, (2, 3, 5, 1, 4, 0, 6))  # k cc hh j hb i w
    return np.ascontiguousarray(x.reshape(2, 128, S * W).astype(np.float16))


def run(inputs, trace=False):
    left = np.ascontiguousarray(np.asarray(inputs["left_gwc"], dtype=np.float32))
    right = np.ascontiguousarray(np.asarray(inputs["right_gwc"], dtype=np.float32))
    lgf = left.reshape(B * G, CG, H, W)
    rgf = right.reshape(B * G, CG, H, W)
    wt = _make_weights()

    in_maps = []
    for c in range(NCORES):
        sl = slice(c * PAIRS, (c + 1) * PAIRS)
        lg_dev = _prep_core(lgf[sl])
        r = _prep_core(rgf[sl]).reshape(2, 128, S, W)
        rg_dev = np.zeros((2, 2, 128, S, PADX), np.float16)
        rg_dev[:, 0, :, :, 48 : 48 + W] = r
        rg_dev[:, 1, :, :, 47 : 47 + W] = r
        in_maps.append(
            {
                "lgd": lg_dev,
                "rgd": np.ascontiguousarray(rg_dev.reshape(2, 2, 128, S * PADX)),
                "wt": wt,
            }
        )

    nc = _get_built()
    res = run_bass_kernel_spmd(nc, in_maps, list(range(NCORES)), trace=trace)

    full = np.empty((B * G, D, H, W), np.float32)
    for c in range(NCORES):
        r = res.results[c]["out"].reshape(D, JW, 32, 2, IW, W)  # d j hh hb i w
        r = np.transpose(r, (4, 1, 0, 3, 2, 5))  # i j d hb hh w
        full[c * PAIRS : (c + 1) * PAIRS] = r.reshape(PAIRS, D, H, W)
    return full.reshape(B, G, D, H, W), res.exec_time_ns


def kernel(**inputs):
    out, _ = run(inputs)
    return out


# revision 13
# speedup vs baseline: 1.1853x; 1.0463x over previous
"""Group-wise correlation cost volume kernel for Trainium2 (8 NeuronCores).

Computes, for inputs left_gwc/right_gwc [B=4, C=320, H=64, W=128] f32:
  cv[b, g, d, h, w] = mean_c( lg[b, g, c, h, w] * rg[b, g, c, h, w-d] )  (0 where w < d)
with G=40 groups (Cg=8 channels each), D=48 disparities.
Output: [4, 40, 48, 64, 128] f32.

Per core (160 (b,g) pairs sharded 20/core across 8 cores):
 - SBUF layout: partitions = (c4, h32); free dim packs 40 slabs
   (j=pair%4, hb=h-block, i=pair//4) x W.  Right features carry 48 zero
   columns of left-padding so the w<d mask falls out as exact zeros, in two
   parity copies (data offset 48 / 47) keeping shifted fp16 reads 4B-aligned
   for the DVE 2x mode.  Host pre-permutes/casts inputs into this layout.
 - Products lg*shift(rg) in fp16 on VectorE (2 big tensor_mul per d).
 - Channel mean on TensorE: static 0.125*block-diag(h32) weights [128,32],
   two accumulating matmuls contract (c4,h32)->h32; 4-way column tiling
   (tile_position (0,32j)) fills all 128 PSUM partitions.
 - PSUM -> SBUF on ScalarE; one HWDGE DMA per d-plane into a scratch-layout
   DRAM output [48, 128, 1280] that the host un-permutes.
"""

import os
import sys

import numpy as np

for _p in ("/opt/trn_rl_repo", "/root/.axon_site/_ro/trn_rl_repo"):
    if os.path.isdir(_p) and _p not in sys.path:
        sys.path.insert(0, _p)

from contextlib import ExitStack

import concourse.bass as bass
import concourse.mybir as mybir
from concourse import bacc
from concourse.bass_utils import run_bass_kernel_spmd
from concourse.tile import TileContext

B, CH, H, W = 4, 320, 64, 128
G, D, CG = 40, 48, 8
NCORES = 8
PAIRS = (B * G) // NCORES  # 20 (b,g) pairs per core
JW = 4                     # column-tile groups
IW = PAIRS // JW           # 5
S = PAIRS * 2              # 40 slabs: s = j*10 + hb*5 + i
PADX = 176                 # 48 pad + 128 data columns of right features
FP16 = mybir.dt.float16
FP32 = mybir.dt.float32

_built = None


def _build():
    nc = bacc.Bacc(
        "TRN2",
        target_bir_lowering=False,
        debug=False,
        num_devices=NCORES,
    )
    lgd = nc.declare_dram_parameter("lgd", [2, 128, S * W], FP16, isOutput=False)
    rgd = nc.declare_dram_parameter("rgd", [2, 2, 128, S * PADX], FP16, isOutput=False)
    wt = nc.declare_dram_parameter("wt", [128, 32], FP16, isOutput=False)
    out = nc.declare_dram_parameter("out", [D, 128, 2 * IW * W], FP32, isOutput=True)

    with TileContext(nc) as tc, ExitStack() as ctx:
        cpool = ctx.enter_context(tc.tile_pool(name="cpool", bufs=1))
        ipool = ctx.enter_context(tc.tile_pool(name="ipool", bufs=1))
        ppool = ctx.enter_context(tc.tile_pool(name="ppool", bufs=6))
        spool = ctx.enter_context(tc.tile_pool(name="spool", bufs=3))
        qpool = ctx.enter_context(tc.tile_pool(name="qpool", bufs=2, space="PSUM"))

        wt_sb = cpool.tile([128, 32], FP16)
        nc.sync.dma_start(out=wt_sb[:], in_=wt[:])

        # Load even-parity tiles, split across the two HWDGE engines so the
        # queues run in parallel; derive the odd-parity (1-shifted) copy
        # on-chip on ScalarE instead of spending input DMA bandwidth on it.
        lg_sb = []  # [k] -> [128, S*W] fp16
        rg_sb = [[None, None], [None, None]]  # [k][parity] -> [128, S*PADX]
        for k in range(2):
            eng = nc.sync if k == 0 else nc.scalar
            lgt = ipool.tile([128, S * W], FP16, name=f"lg{k}", tag=f"lg{k}")
            eng.dma_start(out=lgt[:], in_=lgd[k])
            lg_sb.append(lgt)
            rgt = ipool.tile([128, S * PADX], FP16, name=f"rg{k}0", tag=f"rg{k}0")
            eng.dma_start(out=rgt[:], in_=rgd[k, 0])
            rg_sb[k][0] = rgt
        for k in range(2):
            rgo = ipool.tile([128, S * PADX], FP16, name=f"rg{k}1", tag=f"rg{k}1")
            nc.scalar.copy(rgo[:, 0 : S * PADX - 1], rg_sb[k][0][:, 1 : S * PADX])
            nc.scalar.mul(rgo[:, S * PADX - 1 : S * PADX], rgo[:, S * PADX - 1 : S * PADX], 0.0)
            rg_sb[k][1] = rgo

        lg_v = [t.rearrange("p (s w) -> p s w", w=W) for t in lg_sb]
        rg_v = [
            [t.rearrange("p (s x) -> p s x", x=PADX) for t in row] for row in rg_sb
        ]

        # N<=512 blocks over the 10 slabs (1280 elems) owned by one column group
        UBLK = ((0, 512), (512, 512), (1024, 256))

        for d in range(D):
            par = d & 1
            o = (48 - d) if par == 0 else (47 - d)

            # w < d is structurally zero: skip those products (round the
            # slice start down to even to keep fp16 reads 4B-aligned; the
            # skipped region carries stale garbage that gets zeroed on the
            # staged output below).
            a = d & ~1
            prods = []
            for k in range(2):
                pr = ppool.tile([128, S * W], FP16, name="pr", tag="pr")
                nc.vector.tensor_mul(
                    pr.rearrange("p (s w) -> p s w", w=W)[:, :, a:],
                    lg_v[k][:, :, a:],
                    rg_v[k][par][:, :, o + a : o + W],
                )
                prods.append(pr)

            ps = qpool.tile([128, 1280], FP32, name="ps", tag="ps")
            for j in range(JW):
                for off, n in UBLK:
                    base = j * 10 * W + off
                    for k in range(2):
                        nc.tensor.matmul(
                            ps[32 * j : 32 * j + 32, off : off + n],
                            wt_sb[:],
                            prods[k][:, base : base + n],
                            start=(k == 0),
                            stop=(k == 1),
                            tile_position=(0, 32 * j),
                        )

            st = spool.tile([128, 1280], FP32, name="st", tag="st")
            nc.scalar.copy(st[:], ps[:])
            if d > 0:
                # write-only zeroing (never read the stale w<d region, which
                # may contain non-finite garbage from recycled buffers)
                strip = st.rearrange("p (u w) -> p u w", w=W)[:, :, 0:d]
                nc.gpsimd.memset(strip, 0.0)
            nc.sync.dma_start(out=out[d], in_=st[:])

    nc.compile()
    return nc


def _get_built():
    global _built
    if _built is None:
        _built = _build()
    return _built


def _make_weights():
    wt = np.zeros((128, 32), np.float16)
    for cc in range(4):
        wt[cc * 32 : (cc + 1) * 32, :] = np.eye(32, dtype=np.float16) * np.float16(
            0.125
        )
    return wt


def _prep_core(arr):
    """[20, 8, 64, 128] f32 -> ([2,128,S*W] fp16 partition layout)."""
    x = arr.reshape(IW, JW, 2, 4, 2, 32, W)  # i j k cc hb hh w
    x = np.transpose(x, (2, 3, 5, 1, 4, 0, 6))  # k cc hh j hb i w
    return np.ascontiguousarray(x.reshape(2, 128, S * W).astype(np.float16))


def run(inputs, trace=False):
    left = np.ascontiguousarray(np.asarray(inputs["left_gwc"], dtype=np.float32))
    right = np.ascontiguousarray(np.asarray(inputs["right_gwc"], dtype=np.float32))
    lgf = left.reshape(B * G, CG, H, W)
    rgf = right.reshape(B * G, CG, H, W)
    wt = _make_weights()

    in_maps = []
    for c in range(NCORES):
        sl = slice(c * PAIRS, (c + 1) * PAIRS)
        lg_dev = _prep_core(lgf[sl])
        r = _prep_core(rgf[sl]).reshape(2, 128, S, W)
        rg_dev = np.zeros((2, 2, 128, S, PADX), np.float16)
        rg_dev[:, 0, :, :, 48 : 48 + W] = r
        rg_dev[:, 1, :, :, 47 : 47 + W] = r
        in_maps.append(
            {
                "lgd": lg_dev,
                "rgd": np.ascontiguousarray(rg_dev.reshape(2, 2, 128, S * PADX)),
                "wt": wt,
            }
        )

    nc = _get_built()
    res = run_bass_kernel_spmd(nc, in_maps, list(range(NCORES)), trace=trace)

    full = np.empty((B * G, D, H, W), np.float32)
    for c in range(NCORES):
        r = res.results[c]["out"].reshape(D, JW, 32, 2, IW, W)  # d j hh hb i w
        r = np.transpose(r, (4, 1, 0, 3, 2, 5))  # i j d hb hh w
        full[c * PAIRS : (c + 1) * PAIRS] = r.reshape(PAIRS, D, H, W)
    return full.reshape(B, G, D, H, W), res.exec_time_ns


def kernel(**inputs):
    out, _ = run(inputs)
    return out


# revision 15
# speedup vs baseline: 1.1859x; 1.0005x over previous
"""Group-wise correlation cost volume kernel for Trainium2 (8 NeuronCores).

Computes, for inputs left_gwc/right_gwc [B=4, C=320, H=64, W=128] f32:
  cv[b, g, d, h, w] = mean_c( lg[b, g, c, h, w] * rg[b, g, c, h, w-d] )  (0 where w < d)
with G=40 groups (Cg=8 channels each), D=48 disparities.
Output: [4, 40, 48, 64, 128] f32.

Per core (160 (b,g) pairs sharded 20/core across 8 cores):
 - SBUF layout: partitions = (c4, h32); free dim packs 40 slabs
   (j=pair%4, hb=h-block, i=pair//4) x W.  Right features carry 48 zero
   columns of left-padding so the w<d mask falls out as exact zeros, in two
   parity copies (data offset 48 / 47) keeping shifted fp16 reads 4B-aligned
   for the DVE 2x mode.  Host pre-permutes/casts inputs into this layout.
 - Products lg*shift(rg) in fp16 on VectorE (2 big tensor_mul per d).
 - Channel mean on TensorE: static 0.125*block-diag(h32) weights [128,32],
   two accumulating matmuls contract (c4,h32)->h32; 4-way column tiling
   (tile_position (0,32j)) fills all 128 PSUM partitions.
 - PSUM -> SBUF on ScalarE; one HWDGE DMA per d-plane into a scratch-layout
   DRAM output [48, 128, 1280] that the host un-permutes.
"""

import os
import sys

import numpy as np

for _p in ("/opt/trn_rl_repo", "/root/.axon_site/_ro/trn_rl_repo"):
    if os.path.isdir(_p) and _p not in sys.path:
        sys.path.insert(0, _p)

from contextlib import ExitStack

import concourse.bass as bass
import concourse.mybir as mybir
from concourse import bacc
from concourse.bass_utils import run_bass_kernel_spmd
from concourse.tile import TileContext

B, CH, H, W = 4, 320, 64, 128
G, D, CG = 40, 48, 8
NCORES = 8
PAIRS = (B * G) // NCORES  # 20 (b,g) pairs per core
JW = 4                     # column-tile groups
IW = PAIRS // JW           # 5
S = PAIRS * 2              # 40 slabs: s = j*10 + hb*5 + i
PADX = 176                 # 48 pad + 128 data columns of right features
FP16 = mybir.dt.float16
FP32 = mybir.dt.float32

_built = None


def _build():
    nc = bacc.Bacc(
        "TRN2",
        target_bir_lowering=False,
        debug=False,
        num_devices=NCORES,
    )
    lgd = nc.declare_dram_parameter("lgd", [2, 128, S * W], FP16, isOutput=False)
    rgd = nc.declare_dram_parameter("rgd", [2, 2, 128, S * PADX], FP16, isOutput=False)
    wt = nc.declare_dram_parameter("wt", [128, 32], FP16, isOutput=False)
    out = nc.declare_dram_parameter("out", [D, 128, 2 * IW * W], FP32, isOutput=True)

    with TileContext(nc) as tc, ExitStack() as ctx:
        cpool = ctx.enter_context(tc.tile_pool(name="cpool", bufs=1))
        ipool = ctx.enter_context(tc.tile_pool(name="ipool", bufs=1))
        ppool = ctx.enter_context(tc.tile_pool(name="ppool", bufs=3))
        spool = ctx.enter_context(tc.tile_pool(name="spool", bufs=3))
        qpool = ctx.enter_context(tc.tile_pool(name="qpool", bufs=2, space="PSUM"))

        wt_sb = cpool.tile([128, 32], FP16)
        nc.sync.dma_start(out=wt_sb[:], in_=wt[:])

        # Both channel-halves live in one tile (free dim = (k, slab, w)) so a
        # single DVE op per d covers all products.  Loads are split into slab
        # halves across the two HWDGE engines so d=0 can start early; the
        # odd-parity (1-shifted) copy is derived on-chip on ScalarE instead
        # of spending input DMA bandwidth on it.
        HSLAB = S // 2  # 20 slabs per load half
        lgt = ipool.tile([128, 2 * S * W], FP16, name="lgt")
        lgd_r = lgd.rearrange("k p f -> p k f")
        rgd_r = rgd.rearrange("k q p x -> q p k x")
        for h in range(2):
            fsl = slice(h * HSLAB * W, (h + 1) * HSLAB * W)
            nc.sync.dma_start(
                out=lgt.rearrange("p (kk f) -> p kk f", kk=2)[:, :, fsl],
                in_=lgd_r[:, :, fsl],
            )
        rgt0 = ipool.tile([128, 2 * S * PADX], FP16, name="rgt0")
        rgt0_v = rgt0.rearrange("p (kk f) -> p kk f", kk=2)
        for h in range(2):
            fsl = slice(h * HSLAB * PADX, (h + 1) * HSLAB * PADX)
            nc.scalar.dma_start(out=rgt0_v[:, :, fsl], in_=rgd_r[0][:, :, fsl])
        rgt1 = ipool.tile([128, 2 * S * PADX], FP16, name="rgt1")
        rgt1_v = rgt1.rearrange("p (kk f) -> p kk f", kk=2)
        F = S * PADX
        for kk in range(2):
            nc.scalar.copy(
                rgt1_v[:, kk, 0 : F // 2 - 1], rgt0_v[:, kk, 1 : F // 2]
            )
            nc.scalar.copy(
                rgt1_v[:, kk, F // 2 - 1 : F - 1], rgt0_v[:, kk, F // 2 : F]
            )
            nc.scalar.mul(rgt1_v[:, kk, F - 1 : F], rgt1_v[:, kk, F - 1 : F], 0.0)

        lg_v = lgt.rearrange("p (kk s w) -> p kk s w", kk=2, w=W)
        rg_v = [
            t.rearrange("p (kk s x) -> p kk s x", kk=2, x=PADX) for t in (rgt0, rgt1)
        ]

        # N<=512 blocks over the 10 slabs (1280 elems) owned by one column group
        UBLK = ((0, 512), (512, 512), (1024, 256))

        for d in range(D):
            par = d & 1
            o = (48 - d) if par == 0 else (47 - d)

            # w < d is structurally zero: skip those products (round the
            # slice start down to even to keep fp16 reads 4B-aligned; the
            # skipped region carries stale garbage that gets zeroed on the
            # staged output below).
            a = d & ~1
            pr = ppool.tile([128, 2 * S * W], FP16, name="pr", tag="pr")
            pr_v = pr.rearrange("p (kk s w) -> p kk s w", kk=2, w=W)
            slab_splits = ((0, HSLAB), (HSLAB, S)) if d == 0 else ((0, S),)
            for s0, s1 in slab_splits:
                nc.vector.tensor_mul(
                    pr_v[:, :, s0:s1, a:],
                    lg_v[:, :, s0:s1, a:],
                    rg_v[par][:, :, s0:s1, o + a : o + W],
                )

            ps = qpool.tile([128, 1280], FP32, name="ps", tag="ps")
            for j in range(JW):
                for off, n in UBLK:
                    base = j * 10 * W + off
                    for k in range(2):
                        nc.tensor.matmul(
                            ps[32 * j : 32 * j + 32, off : off + n],
                            wt_sb[:],
                            pr[:, k * S * W + base : k * S * W + base + n],
                            start=(k == 0),
                            stop=(k == 1),
                            tile_position=(0, 32 * j),
                        )

            st = spool.tile([128, 1280], FP32, name="st", tag="st")
            nc.scalar.copy(st[:], ps[:])
            if d > 0:
                # write-only zeroing (never read the stale w<d region, which
                # may contain non-finite garbage from recycled buffers)
                strip = st.rearrange("p (u w) -> p u w", w=W)[:, :, 0:d]
                nc.gpsimd.memset(strip, 0.0)
            nc.sync.dma_start(out=out[d], in_=st[:])

    nc.compile()
    return nc


def _get_built():
    global _built
    if _built is None:
        _built = _build()
    return _built


def _make_weights():
    wt = np.zeros((128, 32), np.float16)
    for cc in range(4):
        wt[cc * 32 : (cc + 1) * 32, :] = np.eye(32, dtype=np.float16) * np.float16(
            0.125
        )
    return wt


def _prep_core(arr):
    """[20, 8, 64, 128] f32 -> ([2,128,S*W] fp16 partition layout)."""
    x = arr.reshape(IW, JW, 2, 4, 2, 32, W)  # i j k cc hb hh w
    x = np.transpose(x, (2, 3, 5, 1, 4, 0, 6))  # k cc hh j hb i w
    return np.ascontiguousarray(x.reshape(2, 128, S * W).astype(np.float16))


def run(inputs, trace=False):
    left = np.ascontiguousarray(np.asarray(inputs["left_gwc"], dtype=np.float32))
    right = np.ascontiguousarray(np.asarray(inputs["right_gwc"], dtype=np.float32))
    lgf = left.reshape(B * G, CG, H, W)
    rgf = right.reshape(B * G, CG, H, W)
    wt = _make_weights()

    in_maps = []
    for c in range(NCORES):
        sl = slice(c * PAIRS, (c + 1) * PAIRS)
        lg_dev = _prep_core(lgf[sl])
        r = _prep_core(rgf[sl]).reshape(2, 128, S, W)
        rg_dev = np.zeros((2, 2, 128, S, PADX), np.float16)
        rg_dev[:, 0, :, :, 48 : 48 + W] = r
        rg_dev[:, 1, :, :, 47 : 47 + W] = r
        in_maps.append(
            {
                "lgd": lg_dev,
                "rgd": np.ascontiguousarray(rg_dev.reshape(2, 2, 128, S * PADX)),
                "wt": wt,
            }
        )

    nc = _get_built()
    res = run_bass_kernel_spmd(nc, in_maps, list(range(NCORES)), trace=trace)

    full = np.empty((B * G, D, H, W), np.float32)
    for c in range(NCORES):
        r = res.results[c]["out"].reshape(D, JW, 32, 2, IW, W)  # d j hh hb i w
        r = np.transpose(r, (4, 1, 0, 3, 2, 5))  # i j d hb hh w
        full[c * PAIRS : (c + 1) * PAIRS] = r.reshape(PAIRS, D, H, W)
    return full.reshape(B, G, D, H, W), res.exec_time_ns


def kernel(**inputs):
    out, _ = run(inputs)
    return out
